# revision 9
# baseline (speedup 1.0000x reference)
"""Trainium2 Bass kernel: 3x3 conv (NCHW 32x256x56x56, 256->256ch, pad 1) with
a host-expanded synthesized weight, data-parallel over 8 NeuronCores.

1D Winograd F(2,3) along x: host de-interleaves the zero-padded image into
even/odd column phases; the device computes the 4 Winograd input planes
V0..V3 with DVE adds (fp16), runs 4 point-GEMMs per output chunk (each
accumulating 2 ci-tiles x 3 dy taps in PSUM, N = 14 rows x 28 tiles = 392),
and reconstructs even/odd output columns with the inverse transform
  out_even = m0 + m1 + m2 + bias,   out_odd = m1 - m2 - m3 + bias
on DVE/GpSimd (bias fused via scalar_tensor_tensor).  This cuts PE matmul
columns 1.5x vs direct conv (2 column-streams per output column instead of
3).  The phase-split output layout is unpermuted on the host.

fp16 operands, fp32 accumulate; all matmul rhs windows are contiguous.
"""

import numpy as np

# Problem constants (hardcoded per contract; kernel.py must be self-contained)
OOC, OIC, K1, K2 = 64, 64, 3, 3
R0, R1 = 4, 4
N_CORES = 8
BATCH = 32
N_PER_CORE = BATCH // N_CORES  # 4
C = 256
H = W = 56
HP = H + 2        # 58 padded rows
TX = 28           # output x-tiles per row (F(2,3): 2 outputs/tile)
EO = 29           # even/odd phase columns (29 each)
FLAT = HP * 2 * EO  # 3364 fp16 elems per channel
RB = 14           # output rows per chunk -> N = RB*TX = 392
NCH = H // RB     # 4 chunks
KT = C // 128     # 2 input-channel tiles
MT = C // 128     # 2 output-channel tiles
NP = 4            # Winograd points
NWIN = RB * TX    # 392 matmul columns

# Input DMA row-bands: first covers chunk 0's rows (+dy halo) so compute
# starts early.
ROW_BANDS = [(0, 19), (19, 20), (39, 19)]

_NC_CACHE = {}
LAST_RESULT = {}  # test.py introspection: last BassKernelResults


def _expand_weight(weight, alphas, betas):
    """W[p0*64+i, p1*64+j, ky, kx] = w[i,j,ky,kx] * a[p0,p1] / (1+exp(w*b[p0,p1]))."""
    w = weight.astype(np.float32)[None, None]            # (1,1,64,64,3,3)
    a = alphas.astype(np.float32).reshape(R0, R1)[:, :, None, None, None, None]
    b = betas.astype(np.float32).reshape(R0, R1)[:, :, None, None, None, None]
    act = w * a / (1.0 + np.exp(w * b))                  # (4,4,64,64,3,3)
    return act.transpose(0, 2, 1, 3, 4, 5).reshape(R0 * OOC, R1 * OIC, K1, K2)


def _host_prep(x, weight, alphas, betas, bias):
    x = np.asarray(x, dtype=np.float32).astype(np.float16)
    xpad = np.pad(x, ((0, 0), (0, 0), (1, 1), (1, 1)))   # (B,C,58,58)
    # de-interleave columns into even/odd phases: (B,C,58,2,29)
    xeo = np.ascontiguousarray(
        xpad.reshape(BATCH, C, HP, EO, 2).transpose(0, 1, 2, 4, 3)
    ).reshape(BATCH, C, FLAT)
    Wfull = _expand_weight(np.asarray(weight), np.asarray(alphas),
                           np.asarray(betas)).astype(np.float32)
    # U[p,dy][ci,co]: Winograd-transformed weights (G w along x)
    w0, w1, w2 = Wfull[:, :, :, 0], Wfull[:, :, :, 1], Wfull[:, :, :, 2]
    U = np.stack([w0, (w0 + w1 + w2) / 2, (w0 - w1 + w2) / 2, w2],
                 axis=0)                                  # (p, co, ci, dy)
    U = U.transpose(2, 0, 3, 1)                           # (ci, p, dy, co)
    # lhsT layout: [ci_local(128), kt, mt, p, dy, co_local(128)]
    w_arr = np.ascontiguousarray(
        U.reshape(KT, 128, NP, K1, MT, 128).transpose(1, 0, 4, 2, 3, 5)
    ).astype(np.float16)
    b_arr = np.ascontiguousarray(
        np.asarray(bias, dtype=np.float32).reshape(MT, 128).T)
    return xeo, w_arr, b_arr


def _build_nc():
    import concourse.mybir as mybir
    import concourse.tile as tile
    from concourse import bacc

    fp32 = mybir.dt.float32
    fp16 = mybir.dt.float16
    add = mybir.AluOpType.add
    sub = mybir.AluOpType.subtract

    nc = bacc.Bacc("TRN2", target_bir_lowering=False, debug=False,
                   num_devices=N_CORES)

    x_d = nc.dram_tensor("x", [N_PER_CORE, C, FLAT], fp16,
                         kind="ExternalInput")
    w_d = nc.dram_tensor("w", [128, KT, MT, NP, K1, 128], fp16,
                         kind="ExternalInput")
    b_d = nc.dram_tensor("b", [128, MT], fp32, kind="ExternalInput")
    # x axis holds (phase, tx) pairs; host unpermutes to interleaved x.
    o_d = nc.dram_tensor("out", [N_PER_CORE, C, H, W], fp32,
                         kind="ExternalOutput")

    def ring(kt):
        return nc.sync if kt == 0 else nc.scalar

    with tile.TileContext(nc) as tc:
        with (
            tc.tile_pool(name="sb", bufs=1) as sb_pool,
            tc.tile_pool(name="ps", bufs=8, space="PSUM") as psum_pool,
        ):
            w_sb = sb_pool.tile([128, KT, MT, NP, K1, 128], fp16,
                                name="w_sb", tag="w_sb")
            b_sb = sb_pool.tile([128, MT], fp32, name="b_sb", tag="b_sb")

            warm_in = sb_pool.tile([128, 128], fp16, name="warm_in",
                                   tag="warm_in")
            nc.vector.memset(warm_in[:], 0.0)

            # Double-buffered phase-split images [rows, phase, tx] and the
            # 4 Winograd V planes per ci-tile.
            xeo = [[sb_pool.tile([128, HP, 2, EO], fp16,
                                 name=f"xeo{par}_{kt}", tag=f"xeo{par}_{kt}")
                    for kt in range(KT)] for par in range(2)]
            vpl = [[[sb_pool.tile([128, HP, TX], fp16,
                                  name=f"v{par}_{kt}_{p}",
                                  tag=f"v{par}_{kt}_{p}")
                     for p in range(NP)] for kt in range(KT)]
                   for par in range(2)]

            xap = x_d.ap()
            oap = o_d.ap()

            def band_dma(n, par, r0, nr):
                for kt in range(KT):
                    ring(kt).dma_start(
                        xeo[par][kt][:, r0:r0 + nr],
                        xap[n, kt * 128:(kt + 1) * 128,
                            r0 * 2 * EO:(r0 + nr) * 2 * EO])

            def forward(par, r0, nr):
                # V planes for padded rows r0..r0+nr (DVE, fp16)
                for kt in range(KT):
                    x_t = xeo[par][kt]
                    e0 = x_t[:, r0:r0 + nr, 0, 0:TX]
                    e1 = x_t[:, r0:r0 + nr, 0, 1:TX + 1]
                    o0 = x_t[:, r0:r0 + nr, 1, 0:TX]
                    o1 = x_t[:, r0:r0 + nr, 1, 1:TX + 1]
                    v = vpl[par][kt]
                    nc.vector.tensor_tensor(v[0][:, r0:r0 + nr], e0, e1, sub)
                    nc.vector.tensor_tensor(v[1][:, r0:r0 + nr], o0, e1, add)
                    nc.vector.tensor_tensor(v[2][:, r0:r0 + nr], e1, o0, sub)
                    nc.vector.tensor_tensor(v[3][:, r0:r0 + nr], o0, o1, sub)

            # PE warmup: junk matmuls bridge the HAM clock-gate window while
            # the first DMAs land.
            # ~3.7us of junk matmuls: enough sustained PE activity to flip
            # the HAM clock gate to 8/8 just as the first real matmul's
            # inputs land (~11.4us).
            warm_ps = psum_pool.tile([128, RB, TX], fp32, name="warm_ps",
                                     tag="m")
            for _ in range(80):
                nc.tensor.matmul(warm_ps[:, 0:2, :], warm_in[:],
                                 warm_in[:, 0:2 * TX])

            # Head DMA order (per ring, sized so each lands just before its
            # first consumer): img0 band0 (first MM ~11.4us), mt0 weights
            # (MMs 0..23), mt1 weights (MMs 24..47 from ~15.5us), img0
            # band2 (chunk 2, ~27us).  img0 band1 (chunk 1, ~19.5us) goes
            # via the idle gpsimd SWDGE ring to dodge the head queue.
            band_dma(0, 0, *ROW_BANDS[0])
            for mt in range(MT):
                for p in range(NP):
                    for kt in range(KT):
                        ring(kt).dma_start(w_sb[:, kt, mt, p],
                                           w_d.ap()[:, kt, mt, p])
            nc.scalar.dma_start(b_sb[:], b_d.ap())
            r0, nr = ROW_BANDS[1]
            for kt in range(KT):
                nc.gpsimd.dma_start(
                    xeo[0][kt][:, r0:r0 + nr],
                    xap[0, kt * 128:(kt + 1) * 128,
                        r0 * 2 * EO:(r0 + nr) * 2 * EO])
            band_dma(0, 0, *ROW_BANDS[2])
            forward(0, *ROW_BANDS[0])
            forward(0, *ROW_BANDS[1])

            # Forward-transform work is drip-fed into the DVE queue between
            # inverse-transform blocks so a not-yet-ready band never blocks
            # the (strict FIFO) queue ahead of PSUM-draining inverse ops.
            def fwd_after_chunk(n, ch):
                if n == 0:
                    sched = {0: (0, 2), 1: (1, 0), 2: (1, 1), 3: (1, 2)}
                else:
                    sched = {1: (n + 1, 0), 2: (n + 1, 1), 3: (n + 1, 2)}
                if ch in sched:
                    img, band = sched[ch]
                    if img < N_PER_CORE:
                        forward(img % 2, *ROW_BANDS[band])

            for n in range(N_PER_CORE):
                par = n % 2
                if n + 1 < N_PER_CORE:
                    for r0, nr in ROW_BANDS:
                        band_dma(n + 1, (n + 1) % 2, r0, nr)
                for ch in range(NCH):
                    y0 = ch * RB
                    for mt in range(MT):
                        m = []
                        for p in range(NP):
                            mp = psum_pool.tile([128, RB, TX], fp32,
                                                name="m", tag="m")
                            m.append(mp)
                            for kt in range(KT):
                                for dy in range(K1):
                                    nc.tensor.matmul(
                                        mp[:, :, :],
                                        w_sb[:, kt, mt, p, dy, :],
                                        vpl[par][kt][p][:, y0 + dy:
                                                        y0 + dy + RB, :],
                                        start=(kt == 0 and dy == 0),
                                        stop=(kt == KT - 1 and dy == K1 - 1),
                                    )
                        # Inverse transform, each op reading <=1 PSUM
                        # operand (HW limit), spread over ACT/DVE/GpSimd:
                        #   out_e = ((m0+bias) + m1) + m2
                        #   out_o = ((m1+bias) - m2) - m3
                        s1 = sb_pool.tile([128, RB, TX], fp32, name="s1",
                                          tag="s1", bufs=3)
                        s2 = sb_pool.tile([128, RB, TX], fp32, name="s2",
                                          tag="s2", bufs=3)
                        r1 = sb_pool.tile([128, RB, TX], fp32, name="r1",
                                          tag="r1", bufs=3)
                        r2 = sb_pool.tile([128, RB, TX], fp32, name="r2",
                                          tag="r2", bufs=3)
                        ot = sb_pool.tile([128, RB, 2, TX], fp32, name="ot",
                                          tag="ot", bufs=4)
                        bias_ap = b_sb[:, mt:mt + 1]
                        nc.scalar.add(s1[:], m[0][:], bias_ap)
                        nc.vector.tensor_tensor(s2[:], s1[:], m[1][:], add)
                        nc.vector.tensor_tensor(ot[:, :, 0, :], s2[:],
                                                m[2][:], add)
                        nc.scalar.add(r1[:], m[1][:], bias_ap)
                        nc.vector.tensor_tensor(r2[:], r1[:], m[2][:], sub)
                        nc.vector.tensor_tensor(ot[:, :, 1, :], r2[:],
                                                m[3][:], sub)
                        dst = oap[n, mt * 128:(mt + 1) * 128, y0:y0 + RB, :]
                        if n == N_PER_CORE - 1 and ch == NCH - 1:
                            half = RB // 2
                            ring(mt).dma_start(dst[:, 0:half, :],
                                               ot[:, 0:half])
                            nc.gpsimd.dma_start(dst[:, half:RB, :],
                                                ot[:, half:RB])
                        else:
                            ring(mt).dma_start(dst, ot[:])
                    fwd_after_chunk(n, ch)
    nc.compile()
    return nc


def get_nc():
    if "nc" not in _NC_CACHE:
        _NC_CACHE["nc"] = _build_nc()
    return _NC_CACHE["nc"]


def kernel(x, weight, alphas, betas, bias):
    from concourse.bass_utils import run_bass_kernel_spmd

    xeo, w_arr, b_arr = _host_prep(x, weight, alphas, betas, bias)
    nc = get_nc()
    in_maps = [
        {"x": xeo[i * N_PER_CORE:(i + 1) * N_PER_CORE], "w": w_arr,
         "b": b_arr}
        for i in range(N_CORES)
    ]
    res = run_bass_kernel_spmd(nc, in_maps, core_ids=list(range(N_CORES)))
    LAST_RESULT["res"] = res
    out = np.concatenate([r["out"] for r in res.results], axis=0)
    # device x-axis is (phase, tx) packed; interleave back to x = 2*tx+phase
    out = out.reshape(BATCH, C, H, 2, TX).transpose(0, 1, 2, 4, 3)
    return np.ascontiguousarray(out).reshape(BATCH, C, H, W)


# revision 11
# speedup vs baseline: 1.0099x; 1.0099x over previous
"""Trainium2 Bass kernel: 3x3 conv (NCHW 32x256x56x56, 256->256ch, pad 1) with
a host-expanded synthesized weight, data-parallel over 8 NeuronCores.

1D Winograd F(2,3) along x: host de-interleaves the zero-padded image into
even/odd column phases; the device computes the 4 Winograd input planes
V0..V3 with DVE adds (fp16), runs 4 point-GEMMs per output chunk (each
accumulating 2 ci-tiles x 3 dy taps in PSUM, N = 14 rows x 28 tiles = 392),
and reconstructs even/odd output columns with the inverse transform
  out_even = m0 + m1 + m2 + bias,   out_odd = m1 - m2 - m3 + bias
on DVE/GpSimd (bias fused via scalar_tensor_tensor).  This cuts PE matmul
columns 1.5x vs direct conv (2 column-streams per output column instead of
3).  The phase-split output layout is unpermuted on the host.

fp16 operands, fp32 accumulate; all matmul rhs windows are contiguous.
"""

import numpy as np

# Problem constants (hardcoded per contract; kernel.py must be self-contained)
OOC, OIC, K1, K2 = 64, 64, 3, 3
R0, R1 = 4, 4
N_CORES = 8
BATCH = 32
N_PER_CORE = BATCH // N_CORES  # 4
C = 256
H = W = 56
HP = H + 2        # 58 padded rows
TX = 28           # output x-tiles per row (F(2,3): 2 outputs/tile)
EO = 29           # even/odd phase columns (29 each)
FLAT = HP * 2 * EO  # 3364 fp16 elems per channel
RB = 14           # output rows per chunk -> N = RB*TX = 392
NCH = H // RB     # 4 chunks
KT = C // 128     # 2 input-channel tiles
MT = C // 128     # 2 output-channel tiles
NP = 4            # Winograd points
NWIN = RB * TX    # 392 matmul columns

# Input DMA row-bands: first covers chunk 0's rows (+dy halo) so compute
# starts early.
ROW_BANDS = [(0, 19), (19, 20), (39, 19)]

_NC_CACHE = {}
LAST_RESULT = {}  # test.py introspection: last BassKernelResults


def _expand_weight(weight, alphas, betas):
    """W[p0*64+i, p1*64+j, ky, kx] = w[i,j,ky,kx] * a[p0,p1] / (1+exp(w*b[p0,p1]))."""
    w = weight.astype(np.float32)[None, None]            # (1,1,64,64,3,3)
    a = alphas.astype(np.float32).reshape(R0, R1)[:, :, None, None, None, None]
    b = betas.astype(np.float32).reshape(R0, R1)[:, :, None, None, None, None]
    act = w * a / (1.0 + np.exp(w * b))                  # (4,4,64,64,3,3)
    return act.transpose(0, 2, 1, 3, 4, 5).reshape(R0 * OOC, R1 * OIC, K1, K2)


def _host_prep(x, weight, alphas, betas, bias):
    x = np.asarray(x, dtype=np.float32).astype(np.float16)
    xpad = np.pad(x, ((0, 0), (0, 0), (1, 1), (1, 1)))   # (B,C,58,58)
    # de-interleave columns into even/odd phases: (B,C,58,2,29)
    xeo = np.ascontiguousarray(
        xpad.reshape(BATCH, C, HP, EO, 2).transpose(0, 1, 2, 4, 3)
    ).reshape(BATCH, C, FLAT)
    Wfull = _expand_weight(np.asarray(weight), np.asarray(alphas),
                           np.asarray(betas)).astype(np.float32)
    # U[p,dy][ci,co]: Winograd-transformed weights (G w along x)
    w0, w1, w2 = Wfull[:, :, :, 0], Wfull[:, :, :, 1], Wfull[:, :, :, 2]
    U = np.stack([w0, (w0 + w1 + w2) / 2, (w0 - w1 + w2) / 2, w2],
                 axis=0)                                  # (p, co, ci, dy)
    U = U.transpose(2, 0, 3, 1)                           # (ci, p, dy, co)
    # lhsT layout: [ci_local(128), kt, mt, p, dy, co_local(128)]
    w_arr = np.ascontiguousarray(
        U.reshape(KT, 128, NP, K1, MT, 128).transpose(1, 0, 4, 2, 3, 5)
    ).astype(np.float16)
    b_arr = np.ascontiguousarray(
        np.asarray(bias, dtype=np.float32).reshape(MT, 128).T)
    return xeo, w_arr, b_arr


def _build_nc():
    import concourse.mybir as mybir
    import concourse.tile as tile
    from concourse import bacc

    fp32 = mybir.dt.float32
    fp16 = mybir.dt.float16
    add = mybir.AluOpType.add
    sub = mybir.AluOpType.subtract

    nc = bacc.Bacc("TRN2", target_bir_lowering=False, debug=False,
                   num_devices=N_CORES)

    x_d = nc.dram_tensor("x", [N_PER_CORE, C, FLAT], fp16,
                         kind="ExternalInput")
    w_d = nc.dram_tensor("w", [128, KT, MT, NP, K1, 128], fp16,
                         kind="ExternalInput")
    b_d = nc.dram_tensor("b", [128, MT], fp32, kind="ExternalInput")
    # x axis holds (phase, tx) pairs; host unpermutes to interleaved x.
    o_d = nc.dram_tensor("out", [N_PER_CORE, C, H, W], fp32,
                         kind="ExternalOutput")

    def ring(kt):
        return nc.sync if kt == 0 else nc.scalar

    with tile.TileContext(nc) as tc:
        with (
            tc.tile_pool(name="sb", bufs=1) as sb_pool,
            tc.tile_pool(name="ps", bufs=8, space="PSUM") as psum_pool,
        ):
            w_sb = sb_pool.tile([128, KT, MT, NP, K1, 128], fp16,
                                name="w_sb", tag="w_sb")
            b_sb = sb_pool.tile([128, MT], fp32, name="b_sb", tag="b_sb")

            warm_in = sb_pool.tile([128, 128], fp16, name="warm_in",
                                   tag="warm_in")
            nc.vector.memset(warm_in[:], 0.0)

            # Double-buffered phase-split images [rows, phase, tx] and the
            # 4 Winograd V planes per ci-tile.
            xeo = [[sb_pool.tile([128, HP, 2, EO], fp16,
                                 name=f"xeo{par}_{kt}", tag=f"xeo{par}_{kt}")
                    for kt in range(KT)] for par in range(2)]
            vpl = [[[sb_pool.tile([128, HP, TX], fp16,
                                  name=f"v{par}_{kt}_{p}",
                                  tag=f"v{par}_{kt}_{p}")
                     for p in range(NP)] for kt in range(KT)]
                   for par in range(2)]

            xap = x_d.ap()
            oap = o_d.ap()

            def band_dma(n, par, r0, nr):
                for kt in range(KT):
                    ring(kt).dma_start(
                        xeo[par][kt][:, r0:r0 + nr],
                        xap[n, kt * 128:(kt + 1) * 128,
                            r0 * 2 * EO:(r0 + nr) * 2 * EO])

            def forward(par, r0, nr):
                # V planes for padded rows r0..r0+nr (DVE, fp16)
                for kt in range(KT):
                    x_t = xeo[par][kt]
                    e0 = x_t[:, r0:r0 + nr, 0, 0:TX]
                    e1 = x_t[:, r0:r0 + nr, 0, 1:TX + 1]
                    o0 = x_t[:, r0:r0 + nr, 1, 0:TX]
                    o1 = x_t[:, r0:r0 + nr, 1, 1:TX + 1]
                    v = vpl[par][kt]
                    nc.vector.tensor_tensor(v[0][:, r0:r0 + nr], e0, e1, sub)
                    nc.vector.tensor_tensor(v[1][:, r0:r0 + nr], o0, e1, add)
                    nc.vector.tensor_tensor(v[2][:, r0:r0 + nr], e1, o0, sub)
                    nc.vector.tensor_tensor(v[3][:, r0:r0 + nr], o0, o1, sub)

            # PE warmup: junk matmuls bridge the HAM clock-gate window while
            # the first DMAs land.
            # ~3.7us of junk matmuls: enough sustained PE activity to flip
            # the HAM clock gate to 8/8 just as the first real matmul's
            # inputs land (~11.4us).
            warm_ps = psum_pool.tile([128, RB, TX], fp32, name="warm_ps",
                                     tag="m")
            for _ in range(80):
                nc.tensor.matmul(warm_ps[:, 0:2, :], warm_in[:],
                                 warm_in[:, 0:2 * TX])

            # Head DMA order (per ring, sized so each lands just before its
            # first consumer): img0 band0 (first MM ~11.4us), mt0 weights
            # (MMs 0..23), mt1 weights (MMs 24..47 from ~15.5us), img0
            # band2 (chunk 2, ~27us).  img0 band1 (chunk 1, ~19.5us) goes
            # via the idle gpsimd SWDGE ring to dodge the head queue.
            # Head DMA order per ring, matched to ~60GB/s ring arrival:
            # img0 band0 (first MMs), mt0 weights (cold-clock MM rate
            # matches their arrival), band1/band2 (chunks 1-2), mt1 weights
            # (first needed ~16us later thanks to the mt-major loop).
            band_dma(0, 0, *ROW_BANDS[0])
            for p in range(NP):
                for kt in range(KT):
                    ring(kt).dma_start(w_sb[:, kt, 0, p],
                                       w_d.ap()[:, kt, 0, p])
            nc.scalar.dma_start(b_sb[:], b_d.ap())
            band_dma(0, 0, *ROW_BANDS[1])
            band_dma(0, 0, *ROW_BANDS[2])
            for p in range(NP):
                for kt in range(KT):
                    ring(kt).dma_start(w_sb[:, kt, 1, p],
                                       w_d.ap()[:, kt, 1, p])
            forward(0, *ROW_BANDS[0])
            forward(0, *ROW_BANDS[1])

            # Forward-transform work is drip-fed into the DVE queue between
            # inverse-transform blocks so a not-yet-ready band never blocks
            # the (strict FIFO) queue ahead of PSUM-draining inverse ops.
            def fwd_after_chunk(n, mt, ch):
                if n == 0 and mt == 0 and ch == 0:
                    forward(0, *ROW_BANDS[2])
                elif mt == 1 and ch in (1, 2, 3) and n + 1 < N_PER_CORE:
                    forward((n + 1) % 2, *ROW_BANDS[ch - 1])

            for n in range(N_PER_CORE):
                par = n % 2
                if n + 1 < N_PER_CORE:
                    for r0, nr in ROW_BANDS:
                        band_dma(n + 1, (n + 1) % 2, r0, nr)
                for mt in range(MT):
                    for ch in range(NCH):
                        y0 = ch * RB
                        m = []
                        for p in range(NP):
                            mp = psum_pool.tile([128, RB, TX], fp32,
                                                name="m", tag="m")
                            m.append(mp)
                            for kt in range(KT):
                                for dy in range(K1):
                                    nc.tensor.matmul(
                                        mp[:, :, :],
                                        w_sb[:, kt, mt, p, dy, :],
                                        vpl[par][kt][p][:, y0 + dy:
                                                        y0 + dy + RB, :],
                                        start=(kt == 0 and dy == 0),
                                        stop=(kt == KT - 1 and dy == K1 - 1),
                                    )
                        # Inverse transform, each op reading <=1 PSUM
                        # operand (HW limit), spread over ACT/DVE/GpSimd:
                        #   out_e = ((m0+bias) + m1) + m2
                        #   out_o = ((m1+bias) - m2) - m3
                        s1 = sb_pool.tile([128, RB, TX], fp32, name="s1",
                                          tag="s1", bufs=3)
                        s2 = sb_pool.tile([128, RB, TX], fp32, name="s2",
                                          tag="s2", bufs=3)
                        r1 = sb_pool.tile([128, RB, TX], fp32, name="r1",
                                          tag="r1", bufs=3)
                        r2 = sb_pool.tile([128, RB, TX], fp32, name="r2",
                                          tag="r2", bufs=3)
                        ot = sb_pool.tile([128, RB, 2, TX], fp32, name="ot",
                                          tag="ot", bufs=4)
                        bias_ap = b_sb[:, mt:mt + 1]
                        nc.scalar.add(s1[:], m[0][:], bias_ap)
                        nc.vector.tensor_tensor(s2[:], s1[:], m[1][:], add)
                        nc.vector.tensor_tensor(ot[:, :, 0, :], s2[:],
                                                m[2][:], add)
                        nc.scalar.add(r1[:], m[1][:], bias_ap)
                        nc.vector.tensor_tensor(r2[:], r1[:], m[2][:], sub)
                        nc.vector.tensor_tensor(ot[:, :, 1, :], r2[:],
                                                m[3][:], sub)
                        dst = oap[n, mt * 128:(mt + 1) * 128, y0:y0 + RB, :]
                        if (n == N_PER_CORE - 1 and mt == MT - 1
                                and ch == NCH - 1):
                            half = RB // 2
                            ring(mt).dma_start(dst[:, 0:half, :],
                                               ot[:, 0:half])
                            nc.gpsimd.dma_start(dst[:, half:RB, :],
                                                ot[:, half:RB])
                        else:
                            ring(mt).dma_start(dst, ot[:])
                        fwd_after_chunk(n, mt, ch)
    nc.compile()
    return nc


def get_nc():
    if "nc" not in _NC_CACHE:
        _NC_CACHE["nc"] = _build_nc()
    return _NC_CACHE["nc"]


def kernel(x, weight, alphas, betas, bias):
    from concourse.bass_utils import run_bass_kernel_spmd

    xeo, w_arr, b_arr = _host_prep(x, weight, alphas, betas, bias)
    nc = get_nc()
    in_maps = [
        {"x": xeo[i * N_PER_CORE:(i + 1) * N_PER_CORE], "w": w_arr,
         "b": b_arr}
        for i in range(N_CORES)
    ]
    res = run_bass_kernel_spmd(nc, in_maps, core_ids=list(range(N_CORES)))
    LAST_RESULT["res"] = res
    out = np.concatenate([r["out"] for r in res.results], axis=0)
    # device x-axis is (phase, tx) packed; interleave back to x = 2*tx+phase
    out = out.reshape(BATCH, C, H, 2, TX).transpose(0, 1, 2, 4, 3)
    return np.ascontiguousarray(out).reshape(BATCH, C, H, W)


# revision 13
# speedup vs baseline: 1.0114x; 1.0014x over previous
"""Trainium2 Bass kernel: 3x3 conv (NCHW 32x256x56x56, 256->256ch, pad 1) with
a host-expanded synthesized weight, data-parallel over 8 NeuronCores.

1D Winograd F(2,3) along x: host de-interleaves the zero-padded image into
even/odd column phases; the device computes the 4 Winograd input planes
V0..V3 with DVE adds (fp16), runs 4 point-GEMMs per output chunk (each
accumulating 2 ci-tiles x 3 dy taps in PSUM, N = 14 rows x 28 tiles = 392),
and reconstructs even/odd output columns with the inverse transform
  out_even = m0 + m1 + m2 + bias,   out_odd = m1 - m2 - m3 + bias
on DVE/GpSimd (bias fused via scalar_tensor_tensor).  This cuts PE matmul
columns 1.5x vs direct conv (2 column-streams per output column instead of
3).  The phase-split output layout is unpermuted on the host.

fp16 operands, fp32 accumulate; all matmul rhs windows are contiguous.
"""

import numpy as np

# Problem constants (hardcoded per contract; kernel.py must be self-contained)
OOC, OIC, K1, K2 = 64, 64, 3, 3
R0, R1 = 4, 4
N_CORES = 8
BATCH = 32
N_PER_CORE = BATCH // N_CORES  # 4
C = 256
H = W = 56
HP = H + 2        # 58 padded rows
TX = 28           # output x-tiles per row (F(2,3): 2 outputs/tile)
EO = 29           # even/odd phase columns (29 each)
FLAT = HP * 2 * EO  # 3364 fp16 elems per channel
RB = 14           # output rows per chunk -> N = RB*TX = 392
NCH = H // RB     # 4 chunks
KT = C // 128     # 2 input-channel tiles
MT = C // 128     # 2 output-channel tiles
NP = 4            # Winograd points
NWIN = RB * TX    # 392 matmul columns

# Input DMA row-bands: first covers chunk 0's rows (+dy halo) so compute
# starts early.
ROW_BANDS = [(0, 19), (19, 20), (39, 19)]

_NC_CACHE = {}
LAST_RESULT = {}  # test.py introspection: last BassKernelResults


def _expand_weight(weight, alphas, betas):
    """W[p0*64+i, p1*64+j, ky, kx] = w[i,j,ky,kx] * a[p0,p1] / (1+exp(w*b[p0,p1]))."""
    w = weight.astype(np.float32)[None, None]            # (1,1,64,64,3,3)
    a = alphas.astype(np.float32).reshape(R0, R1)[:, :, None, None, None, None]
    b = betas.astype(np.float32).reshape(R0, R1)[:, :, None, None, None, None]
    act = w * a / (1.0 + np.exp(w * b))                  # (4,4,64,64,3,3)
    return act.transpose(0, 2, 1, 3, 4, 5).reshape(R0 * OOC, R1 * OIC, K1, K2)


def _host_prep(x, weight, alphas, betas, bias):
    x = np.asarray(x, dtype=np.float32).astype(np.float16)
    xpad = np.pad(x, ((0, 0), (0, 0), (1, 1), (1, 1)))   # (B,C,58,58)
    # de-interleave columns into even/odd phases: (B,C,58,2,29)
    xeo = np.ascontiguousarray(
        xpad.reshape(BATCH, C, HP, EO, 2).transpose(0, 1, 2, 4, 3)
    ).reshape(BATCH, C, FLAT)
    Wfull = _expand_weight(np.asarray(weight), np.asarray(alphas),
                           np.asarray(betas)).astype(np.float32)
    # U[p,dy][ci,co]: Winograd-transformed weights (G w along x)
    w0, w1, w2 = Wfull[:, :, :, 0], Wfull[:, :, :, 1], Wfull[:, :, :, 2]
    U = np.stack([w0, (w0 + w1 + w2) / 2, (w0 - w1 + w2) / 2, w2],
                 axis=0)                                  # (p, co, ci, dy)
    U = U.transpose(2, 0, 3, 1)                           # (ci, p, dy, co)
    # lhsT layout: [ci_local(128), kt, mt, p, dy, co_local(128)]
    w_arr = np.ascontiguousarray(
        U.reshape(KT, 128, NP, K1, MT, 128).transpose(1, 0, 4, 2, 3, 5)
    ).astype(np.float16)
    b_arr = np.ascontiguousarray(
        np.asarray(bias, dtype=np.float32).reshape(MT, 128).T)
    return xeo, w_arr, b_arr


def _build_nc():
    import concourse.mybir as mybir
    import concourse.tile as tile
    from concourse import bacc

    fp32 = mybir.dt.float32
    fp16 = mybir.dt.float16
    add = mybir.AluOpType.add
    sub = mybir.AluOpType.subtract

    nc = bacc.Bacc("TRN2", target_bir_lowering=False, debug=False,
                   num_devices=N_CORES)

    x_d = nc.dram_tensor("x", [N_PER_CORE, C, FLAT], fp16,
                         kind="ExternalInput")
    w_d = nc.dram_tensor("w", [128, KT, MT, NP, K1, 128], fp16,
                         kind="ExternalInput")
    b_d = nc.dram_tensor("b", [128, MT], fp32, kind="ExternalInput")
    # x axis holds (phase, tx) pairs; host unpermutes to interleaved x.
    o_d = nc.dram_tensor("out", [N_PER_CORE, C, H, W], fp32,
                         kind="ExternalOutput")

    def ring(kt):
        return nc.sync if kt == 0 else nc.scalar

    with tile.TileContext(nc) as tc:
        with (
            tc.tile_pool(name="sb", bufs=1) as sb_pool,
            tc.tile_pool(name="ps", bufs=8, space="PSUM") as psum_pool,
        ):
            w_sb = sb_pool.tile([128, KT, MT, NP, K1, 128], fp16,
                                name="w_sb", tag="w_sb")
            b_sb = sb_pool.tile([128, MT], fp32, name="b_sb", tag="b_sb")

            warm_in = sb_pool.tile([128, 128], fp16, name="warm_in",
                                   tag="warm_in")
            nc.vector.memset(warm_in[:], 0.0)

            # Double-buffered phase-split images [rows, phase, tx] and the
            # 4 Winograd V planes per ci-tile.
            xeo = [[sb_pool.tile([128, HP, 2, EO], fp16,
                                 name=f"xeo{par}_{kt}", tag=f"xeo{par}_{kt}")
                    for kt in range(KT)] for par in range(2)]
            vpl = [[[sb_pool.tile([128, HP, TX], fp16,
                                  name=f"v{par}_{kt}_{p}",
                                  tag=f"v{par}_{kt}_{p}")
                     for p in range(NP)] for kt in range(KT)]
                   for par in range(2)]

            xap = x_d.ap()
            oap = o_d.ap()

            def band_dma(n, par, r0, nr):
                for kt in range(KT):
                    ring(kt).dma_start(
                        xeo[par][kt][:, r0:r0 + nr],
                        xap[n, kt * 128:(kt + 1) * 128,
                            r0 * 2 * EO:(r0 + nr) * 2 * EO])

            def forward(par, r0, nr):
                # V planes for padded rows r0..r0+nr (DVE, fp16)
                for kt in range(KT):
                    x_t = xeo[par][kt]
                    e0 = x_t[:, r0:r0 + nr, 0, 0:TX]
                    e1 = x_t[:, r0:r0 + nr, 0, 1:TX + 1]
                    o0 = x_t[:, r0:r0 + nr, 1, 0:TX]
                    o1 = x_t[:, r0:r0 + nr, 1, 1:TX + 1]
                    v = vpl[par][kt]
                    nc.vector.tensor_tensor(v[0][:, r0:r0 + nr], e0, e1, sub)
                    nc.vector.tensor_tensor(v[1][:, r0:r0 + nr], o0, e1, add)
                    nc.vector.tensor_tensor(v[2][:, r0:r0 + nr], e1, o0, sub)
                    nc.vector.tensor_tensor(v[3][:, r0:r0 + nr], o0, o1, sub)

            # PE warmup: junk matmuls start the HAM busy window while the
            # first DMAs land; the cold-clock first chunk then consumes mt0
            # weights at roughly their ~60GB/s-per-ring arrival rate.
            warm_ps = psum_pool.tile([128, RB, TX], fp32, name="warm_ps",
                                     tag="m")
            for _ in range(20):
                nc.tensor.matmul(warm_ps[:, 0, :], warm_in[:],
                                 warm_in[:, 0:TX])

            # Head: image-0 band0 + mt0 weights first so the first GEMM
            # chain waits on the fewest bytes; image bands next (chunks 1-2
            # consume them before mt1 starts); mt1 weights last.
            band_dma(0, 0, *ROW_BANDS[0])
            for p in range(NP):
                for kt in range(KT):
                    ring(kt).dma_start(w_sb[:, kt, 0, p], w_d.ap()[:, kt, 0, p])
            nc.scalar.dma_start(b_sb[:], b_d.ap())
            forward(0, *ROW_BANDS[0])
            for r0, nr in ROW_BANDS[1:]:
                band_dma(0, 0, r0, nr)
                forward(0, r0, nr)
            for p in range(NP):
                for kt in range(KT):
                    ring(kt).dma_start(w_sb[:, kt, 1, p], w_d.ap()[:, kt, 1, p])

            for n in range(N_PER_CORE):
                par = n % 2
                if n > 0:
                    for r0, nr in ROW_BANDS:
                        band_dma(n, par, r0, nr)
                        forward(par, r0, nr)
                for ch in range(NCH):
                    y0 = ch * RB
                    for mt in range(MT):
                        m = []
                        for p in range(NP):
                            mp = psum_pool.tile([128, RB, TX], fp32,
                                                name="m", tag="m")
                            m.append(mp)
                            for kt in range(KT):
                                for dy in range(K1):
                                    nc.tensor.matmul(
                                        mp[:, :, :],
                                        w_sb[:, kt, mt, p, dy, :],
                                        vpl[par][kt][p][:, y0 + dy:
                                                        y0 + dy + RB, :],
                                        start=(kt == 0 and dy == 0),
                                        stop=(kt == KT - 1 and dy == K1 - 1),
                                    )
                        # Inverse transform, each op reading <=1 PSUM
                        # operand (HW limit), spread over ACT/DVE/GpSimd:
                        #   out_e = ((m0+bias) + m1) + m2
                        #   out_o = ((m1+bias) - m2) - m3
                        s1 = sb_pool.tile([128, RB, TX], fp32, name="s1",
                                          tag="s1", bufs=3)
                        s2 = sb_pool.tile([128, RB, TX], fp32, name="s2",
                                          tag="s2", bufs=3)
                        r1 = sb_pool.tile([128, RB, TX], fp32, name="r1",
                                          tag="r1", bufs=3)
                        r2 = sb_pool.tile([128, RB, TX], fp32, name="r2",
                                          tag="r2", bufs=3)
                        ot = sb_pool.tile([128, RB, 2, TX], fp32, name="ot",
                                          tag="ot", bufs=4)
                        bias_ap = b_sb[:, mt:mt + 1]
                        nc.scalar.add(s1[:], m[0][:], bias_ap)
                        nc.vector.tensor_tensor(s2[:], s1[:], m[1][:], add)
                        nc.vector.tensor_tensor(ot[:, :, 0, :], s2[:],
                                                m[2][:], add)
                        nc.scalar.add(r1[:], m[1][:], bias_ap)
                        nc.vector.tensor_tensor(r2[:], r1[:], m[2][:], sub)
                        nc.vector.tensor_tensor(ot[:, :, 1, :], r2[:],
                                                m[3][:], sub)
                        dst = oap[n, mt * 128:(mt + 1) * 128, y0:y0 + RB, :]
                        if n == N_PER_CORE - 1 and ch == NCH - 1:
                            half = RB // 2
                            ring(mt).dma_start(dst[:, 0:half, :],
                                               ot[:, 0:half])
                            nc.gpsimd.dma_start(dst[:, half:RB, :],
                                                ot[:, half:RB])
                        else:
                            ring(mt).dma_start(dst, ot[:])
    nc.compile()
    return nc


def get_nc():
    if "nc" not in _NC_CACHE:
        _NC_CACHE["nc"] = _build_nc()
    return _NC_CACHE["nc"]


def kernel(x, weight, alphas, betas, bias):
    from concourse.bass_utils import run_bass_kernel_spmd

    xeo, w_arr, b_arr = _host_prep(x, weight, alphas, betas, bias)
    nc = get_nc()
    in_maps = [
        {"x": xeo[i * N_PER_CORE:(i + 1) * N_PER_CORE], "w": w_arr,
         "b": b_arr}
        for i in range(N_CORES)
    ]
    res = run_bass_kernel_spmd(nc, in_maps, core_ids=list(range(N_CORES)))
    LAST_RESULT["res"] = res
    out = np.concatenate([r["out"] for r in res.results], axis=0)
    # device x-axis is (phase, tx) packed; interleave back to x = 2*tx+phase
    out = out.reshape(BATCH, C, H, 2, TX).transpose(0, 1, 2, 4, 3)
    return np.ascontiguousarray(out).reshape(BATCH, C, H, W)


# revision 17
# speedup vs baseline: 1.0116x; 1.0003x over previous
"""Trainium2 Bass kernel: 3x3 conv (NCHW 32x256x56x56, 256->256ch, pad 1) with
a host-expanded synthesized weight, data-parallel over 8 NeuronCores.

1D Winograd F(2,3) along x: host de-interleaves the zero-padded image into
even/odd column phases; the device computes the 4 Winograd input planes
V0..V3 with DVE adds (fp16), runs 4 point-GEMMs per output chunk (each
accumulating 2 ci-tiles x 3 dy taps in PSUM, N = 14 rows x 28 tiles = 392),
and reconstructs even/odd output columns with the inverse transform
  out_even = m0 + m1 + m2 + bias,   out_odd = m1 - m2 - m3 + bias
on DVE/GpSimd (bias fused via scalar_tensor_tensor).  This cuts PE matmul
columns 1.5x vs direct conv (2 column-streams per output column instead of
3).  The phase-split output layout is unpermuted on the host.

fp16 operands, fp32 accumulate; all matmul rhs windows are contiguous.
"""

import numpy as np

# Problem constants (hardcoded per contract; kernel.py must be self-contained)
OOC, OIC, K1, K2 = 64, 64, 3, 3
R0, R1 = 4, 4
N_CORES = 8
BATCH = 32
N_PER_CORE = BATCH // N_CORES  # 4
C = 256
H = W = 56
HP = H + 2        # 58 padded rows
TX = 28           # output x-tiles per row (F(2,3): 2 outputs/tile)
EO = 29           # even/odd phase columns (29 each)
FLAT = HP * 2 * EO  # 3364 fp16 elems per channel
RB = 14           # output rows per chunk -> N = RB*TX = 392
NCH = H // RB     # 4 chunks
KT = C // 128     # 2 input-channel tiles
MT = C // 128     # 2 output-channel tiles
NP = 4            # Winograd points
NWIN = RB * TX    # 392 matmul columns

# Input DMA row-bands: first covers chunk 0's rows (+dy halo) so compute
# starts early.
ROW_BANDS = [(0, 19), (19, 20), (39, 19)]

_NC_CACHE = {}
LAST_RESULT = {}  # test.py introspection: last BassKernelResults


def _expand_weight(weight, alphas, betas):
    """W[p0*64+i, p1*64+j, ky, kx] = w[i,j,ky,kx] * a[p0,p1] / (1+exp(w*b[p0,p1]))."""
    w = weight.astype(np.float32)[None, None]            # (1,1,64,64,3,3)
    a = alphas.astype(np.float32).reshape(R0, R1)[:, :, None, None, None, None]
    b = betas.astype(np.float32).reshape(R0, R1)[:, :, None, None, None, None]
    act = w * a / (1.0 + np.exp(w * b))                  # (4,4,64,64,3,3)
    return act.transpose(0, 2, 1, 3, 4, 5).reshape(R0 * OOC, R1 * OIC, K1, K2)


def _host_prep(x, weight, alphas, betas, bias):
    x = np.asarray(x, dtype=np.float32).astype(np.float16)
    xpad = np.pad(x, ((0, 0), (0, 0), (1, 1), (1, 1)))   # (B,C,58,58)
    # de-interleave columns into even/odd phases: (B,C,58,2,29)
    xeo = np.ascontiguousarray(
        xpad.reshape(BATCH, C, HP, EO, 2).transpose(0, 1, 2, 4, 3)
    ).reshape(BATCH, C, FLAT)
    Wfull = _expand_weight(np.asarray(weight), np.asarray(alphas),
                           np.asarray(betas)).astype(np.float32)
    # U[p,dy][ci,co]: Winograd-transformed weights (G w along x)
    w0, w1, w2 = Wfull[:, :, :, 0], Wfull[:, :, :, 1], Wfull[:, :, :, 2]
    U = np.stack([w0, (w0 + w1 + w2) / 2, (w0 - w1 + w2) / 2, w2],
                 axis=0)                                  # (p, co, ci, dy)
    U = U.transpose(2, 0, 3, 1)                           # (ci, p, dy, co)
    # lhsT layout: [ci_local(128), kt, mt, p, dy, co_local(128)]
    w_arr = np.ascontiguousarray(
        U.reshape(KT, 128, NP, K1, MT, 128).transpose(1, 0, 4, 2, 3, 5)
    ).astype(np.float16)
    b_arr = np.ascontiguousarray(
        np.asarray(bias, dtype=np.float32).reshape(MT, 128).T)
    return xeo, w_arr, b_arr


def _build_nc():
    import concourse.mybir as mybir
    import concourse.tile as tile
    from concourse import bacc

    fp32 = mybir.dt.float32
    fp16 = mybir.dt.float16
    add = mybir.AluOpType.add
    sub = mybir.AluOpType.subtract

    nc = bacc.Bacc("TRN2", target_bir_lowering=False, debug=False,
                   num_devices=N_CORES)

    x_d = nc.dram_tensor("x", [N_PER_CORE, C, FLAT], fp16,
                         kind="ExternalInput")
    w_d = nc.dram_tensor("w", [128, KT, MT, NP, K1, 128], fp16,
                         kind="ExternalInput")
    b_d = nc.dram_tensor("b", [128, MT], fp32, kind="ExternalInput")
    # x axis holds (phase, tx) pairs; host unpermutes to interleaved x.
    o_d = nc.dram_tensor("out", [N_PER_CORE, C, H, W], fp32,
                         kind="ExternalOutput")

    def ring(kt):
        return nc.sync if kt == 0 else nc.scalar

    with tile.TileContext(nc) as tc:
        with (
            tc.tile_pool(name="sb", bufs=1) as sb_pool,
            tc.tile_pool(name="ps", bufs=8, space="PSUM") as psum_pool,
        ):
            w_sb = sb_pool.tile([128, KT, MT, NP, K1, 128], fp16,
                                name="w_sb", tag="w_sb")
            b_sb = sb_pool.tile([128, MT], fp32, name="b_sb", tag="b_sb")

            warm_in = sb_pool.tile([128, 128], fp16, name="warm_in",
                                   tag="warm_in")
            nc.vector.memset(warm_in[:], 0.0)

            # Double-buffered phase-split images [rows, phase, tx] and the
            # 4 Winograd V planes per ci-tile.
            xeo = [[sb_pool.tile([128, HP, 2, EO], fp16,
                                 name=f"xeo{par}_{kt}", tag=f"xeo{par}_{kt}")
                    for kt in range(KT)] for par in range(2)]
            # One tile per (par, kt) holding all 4 Winograd planes: fewer
            # tile tags -> fewer framework semaphores in the pre/postamble.
            vpl = [[sb_pool.tile([128, NP, HP, TX], fp16,
                                 name=f"v{par}_{kt}", tag=f"v{par}_{kt}")
                    for kt in range(KT)] for par in range(2)]

            xap = x_d.ap()
            oap = o_d.ap()

            def band_dma(n, par, r0, nr):
                for kt in range(KT):
                    ring(kt).dma_start(
                        xeo[par][kt][:, r0:r0 + nr],
                        xap[n, kt * 128:(kt + 1) * 128,
                            r0 * 2 * EO:(r0 + nr) * 2 * EO])

            def forward(par, r0, nr):
                # V planes for padded rows r0..r0+nr (DVE, fp16)
                for kt in range(KT):
                    x_t = xeo[par][kt]
                    e0 = x_t[:, r0:r0 + nr, 0, 0:TX]
                    e1 = x_t[:, r0:r0 + nr, 0, 1:TX + 1]
                    o0 = x_t[:, r0:r0 + nr, 1, 0:TX]
                    o1 = x_t[:, r0:r0 + nr, 1, 1:TX + 1]
                    v = vpl[par][kt]
                    nc.vector.tensor_tensor(v[:, 0, r0:r0 + nr], e0, e1, sub)
                    nc.vector.tensor_tensor(v[:, 1, r0:r0 + nr], o0, e1, add)
                    nc.vector.tensor_tensor(v[:, 2, r0:r0 + nr], e1, o0, sub)
                    nc.vector.tensor_tensor(v[:, 3, r0:r0 + nr], o0, o1, sub)

            # PE warmup: junk matmuls start the HAM busy window while the
            # first DMAs land; the cold-clock first chunk then consumes mt0
            # weights at roughly their ~60GB/s-per-ring arrival rate.
            warm_ps = psum_pool.tile([128, RB, TX], fp32, name="warm_ps",
                                     tag="m")
            for _ in range(20):
                nc.tensor.matmul(warm_ps[:, 0, :], warm_in[:],
                                 warm_in[:, 0:TX])

            # Head: image-0 band0 + mt0 weights first so the first GEMM
            # chain waits on the fewest bytes; image bands next (chunks 1-2
            # consume them before mt1 starts); mt1 weights last.
            band_dma(0, 0, *ROW_BANDS[0])
            for p in range(NP):
                for kt in range(KT):
                    ring(kt).dma_start(w_sb[:, kt, 0, p], w_d.ap()[:, kt, 0, p])
            nc.scalar.dma_start(b_sb[:], b_d.ap())
            forward(0, *ROW_BANDS[0])
            for r0, nr in ROW_BANDS[1:]:
                band_dma(0, 0, r0, nr)
                forward(0, r0, nr)
            for p in range(NP):
                for kt in range(KT):
                    ring(kt).dma_start(w_sb[:, kt, 1, p], w_d.ap()[:, kt, 1, p])

            for n in range(N_PER_CORE):
                par = n % 2
                if n > 0:
                    for r0, nr in ROW_BANDS:
                        band_dma(n, par, r0, nr)
                        forward(par, r0, nr)
                for ch in range(NCH):
                    y0 = ch * RB
                    for mt in range(MT):
                        m = []
                        for p in range(NP):
                            mp = psum_pool.tile([128, RB, TX], fp32,
                                                name="m", tag="m")
                            m.append(mp)
                            for kt in range(KT):
                                for dy in range(K1):
                                    nc.tensor.matmul(
                                        mp[:, :, :],
                                        w_sb[:, kt, mt, p, dy, :],
                                        vpl[par][kt][:, p, y0 + dy:
                                                     y0 + dy + RB, :],
                                        start=(kt == 0 and dy == 0),
                                        stop=(kt == KT - 1 and dy == K1 - 1),
                                    )
                        # Inverse transform, each op reading <=1 PSUM
                        # operand (HW limit), spread over ACT/DVE/GpSimd:
                        #   out_e = ((m0+bias) + m1) + m2
                        #   out_o = ((m1+bias) - m2) - m3
                        s1 = sb_pool.tile([128, RB, TX], fp32, name="s1",
                                          tag="s1", bufs=3)
                        s2 = sb_pool.tile([128, RB, TX], fp32, name="s2",
                                          tag="s2", bufs=3)
                        r1 = sb_pool.tile([128, RB, TX], fp32, name="r1",
                                          tag="r1", bufs=3)
                        r2 = sb_pool.tile([128, RB, TX], fp32, name="r2",
                                          tag="r2", bufs=3)
                        ot = sb_pool.tile([128, RB, 2, TX], fp32, name="ot",
                                          tag="ot", bufs=4)
                        bias_ap = b_sb[:, mt:mt + 1]
                        nc.scalar.add(s1[:], m[0][:], bias_ap)
                        nc.vector.tensor_tensor(s2[:], s1[:], m[1][:], add)
                        nc.vector.tensor_tensor(ot[:, :, 0, :], s2[:],
                                                m[2][:], add)
                        nc.scalar.add(r1[:], m[1][:], bias_ap)
                        nc.vector.tensor_tensor(r2[:], r1[:], m[2][:], sub)
                        nc.vector.tensor_tensor(ot[:, :, 1, :], r2[:],
                                                m[3][:], sub)
                        dst = oap[n, mt * 128:(mt + 1) * 128, y0:y0 + RB, :]
                        if n == N_PER_CORE - 1 and ch == NCH - 1:
                            # 9/5 row split: gpsimd carries both mt tails,
                            # so give the HWDGE rings the bigger share.
                            cut = 9
                            ring(mt).dma_start(dst[:, 0:cut, :],
                                               ot[:, 0:cut])
                            nc.gpsimd.dma_start(dst[:, cut:RB, :],
                                                ot[:, cut:RB])
                        else:
                            ring(mt).dma_start(dst, ot[:])
    nc.compile()
    return nc


def get_nc():
    if "nc" not in _NC_CACHE:
        _NC_CACHE["nc"] = _build_nc()
    return _NC_CACHE["nc"]


def kernel(x, weight, alphas, betas, bias):
    from concourse.bass_utils import run_bass_kernel_spmd

    xeo, w_arr, b_arr = _host_prep(x, weight, alphas, betas, bias)
    nc = get_nc()
    in_maps = [
        {"x": xeo[i * N_PER_CORE:(i + 1) * N_PER_CORE], "w": w_arr,
         "b": b_arr}
        for i in range(N_CORES)
    ]
    res = run_bass_kernel_spmd(nc, in_maps, core_ids=list(range(N_CORES)))
    LAST_RESULT["res"] = res
    out = np.concatenate([r["out"] for r in res.results], axis=0)
    # device x-axis is (phase, tx) packed; interleave back to x = 2*tx+phase
    out = out.reshape(BATCH, C, H, 2, TX).transpose(0, 1, 2, 4, 3)
    return np.ascontiguousarray(out).reshape(BATCH, C, H, W)
